# revision 25
# baseline (speedup 1.0000x reference)
"""NextVLAD TRN2 kernel v3 — token-major (m-on-partitions) dataflow.

8-way data-parallel (1 sample/core, M=512 tokens each). Host combines the
FC weights so every PE matmul contracts directly over the input feature
dim n=1024:
  W_all = [W_inp^T | (W_gk@W_inp)^T]            (fp8; x fp8)
  x_aug = [x m-block | (W_g@W_inp)^T replica]   (gates ride the Gram rhs)
Per m-tile (128 tokens) fp8-DoubleRow chains produce y (2048) and gk
logits (1024) with tokens on PSUM partitions — no PE transposes of y.
A per-m-tile Gram matmul (x^T [x|Wg]) gives sum-of-squares (diagonal via
masked tensor_tensor_reduce) plus the gate logits in the same pass;
inv_m = exp(-0.5*ln(ss) - ln(128)) on the scalar engine (single act
table: ln/exp/square/copy).
yT g-blocks hold sg[m,g]*inv[m]*y (bias-free, bf16); col D is 1.0 and
col D+1 is sg[m,g]. The VLAD einsum uses lhsT = ex directly, so per
group g the PSUM accumulator vd_g carries:
  cols 0..D-1: sum_m ex*sg*yhat | col D: se (softmax denom!) | D+1: Sg
Softmax division is deferred: V = sum_g (1/se_g) * vd_g. b_inp re-enters
analytically: Vc = isg^T @ binp_mat via one fp32 PE matmul, with
isg_g = Sg/se_g. vlad = V + Vc - T*cent, then fused square+reduce and
ln/exp rsqrt for the final l2 normalization.
"""
import math
import os
import numpy as np

N = 1024          # feature size
EN = 2048         # expanded features
G = 8             # groups
KC = 128          # clusters
D = 256           # per-group cluster dim
BW = D + 2        # einsum group block width (data | ones | sg)
M = 512           # tokens per sample
MT = 4            # m-tiles of 128
C = 4             # contraction chunk-pairs (256 rows each, DoubleRow)
XG = 192          # x-aug group: 128 x cols + 64 gate-weight cols
SX = 8.0          # x fp8 scale
SW = 128.0        # W fp8 scale
LN128 = math.log(128.0)

_cache = {}


def _build_nc():
    import concourse.bacc as bacc
    import concourse.tile as tile
    from concourse import mybir

    f32 = mybir.dt.float32
    f32r = mybir.dt.float32r
    bf16 = mybir.dt.bfloat16
    fp8 = mybir.dt.float8e4
    Alu = mybir.AluOpType
    Act = mybir.ActivationFunctionType
    DR = mybir.MatmulPerfMode.DoubleRow

    nc = bacc.Bacc("TRN2", target_bir_lowering=False)
    x_d = nc.dram_tensor("x", [128, C * 2 * MT * XG], fp8, kind="ExternalInput")
    w_d = nc.dram_tensor("w", [128, 6 * 4096], fp8, kind="ExternalInput")
    binpm_d = nc.dram_tensor("binpm", [G, D], f32, kind="ExternalInput")
    cd_d = nc.dram_tensor("cd", [128, 272], f32, kind="ExternalInput")
    idf_d = nc.dram_tensor("idf", [128, 128], f32, kind="ExternalInput")
    idb_d = nc.dram_tensor("idb", [128, 128], bf16, kind="ExternalInput")
    out_d = nc.dram_tensor("out", [KC, D], f32, kind="ExternalOutput")

    with tile.TileContext(nc) as tc:
        with tc.tile_pool(name="const", bufs=1) as constp, \
             tc.tile_pool(name="persist", bufs=1) as persist, \
             tc.tile_pool(name="work", bufs=1) as work, \
             tc.tile_pool(name="fin", bufs=1) as fin, \
             tc.tile_pool(name="psm", bufs=1, space="PSUM") as psm:

            # ---------------- DMAs ----------------
            # big stream on the gpsimd queue (cheap dispatch), consts on sync
            x_t = constp.tile([128, C * 2 * MT * XG], fp8, name="x")
            nc.gpsimd.dma_start(out=x_t[:], in_=x_d[:])
            w_t = [constp.tile([128, 4096], fp8, name=f"w{j}") for j in range(6)]
            for j in (4, 5, 0, 1, 2, 3):  # gk slices first, then y slices
                nc.gpsimd.dma_start(out=w_t[j][:],
                                    in_=w_d[:, j * 4096:(j + 1) * 4096])
            idf_t = constp.tile([128, 128], f32, name="idf")
            nc.sync.dma_start(out=idf_t[:], in_=idf_d[:])
            idb_t = constp.tile([128, 128], bf16, name="idb")
            nc.sync.dma_start(out=idb_t[:], in_=idb_d[:])
            cd_t = constp.tile([128, 272], f32, name="cd")
            nc.sync.dma_start(out=cd_t[:], in_=cd_d[:])
            binpm_t = constp.tile([G, D], f32, name="binpm")
            nc.sync.dma_start(out=binpm_t[:], in_=binpm_d[:])

            xv = x_t.rearrange("p (c s q) -> p c s q", c=C, s=2)
            wv = [t.rearrange("p (c s w) -> p c s w", c=C, s=2) for t in w_t]

            # persistent data
            yT = [persist.tile([128, G * BW], bf16, name=f"yT{m}") for m in range(MT)]
            yT3 = [t.rearrange("p (g c) -> p g c", c=BW) for t in yT]
            ex_t = [persist.tile([128, G * KC], bf16, name=f"ex{m}") for m in range(MT)]
            inv_t = persist.tile([128, MT], f32, name="inv")
            ninv_t = persist.tile([128, MT], f32, name="ninv")
            sg_t = persist.tile([128, MT * G], f32, name="sg")
            sgi_t = persist.tile([128, MT * G], f32, name="sgi")
            iseT_t = persist.tile([128, G], f32, name="iseT")
            isg_t = persist.tile([128, G], f32, name="isg")

            # einsum const cols: col D = 1.0 (-> se); col D+1 = sg (-> Sg)
            for m in range(MT):
                nc.vector.memset(yT3[m][:, :, D:D + 1], 1.0)

            # ---- Gram chains: ss (diag) + gate logits in one pass ----
            gsc_t = work.tile([128, MT * 128], f32, name="gsc")
            ssT_t = work.tile([128, MT], f32, name="ssT")
            gt_t = work.tile([128, MT * G], f32, name="gt")
            graw_t = work.tile([128, MT * G], f32, name="graw")
            for m in range(MT):
                g_ps = psm.tile([128, XG], f32, name="gram", tag="tp", bufs=2)
                for c in range(C):
                    nc.tensor.matmul(g_ps[:],
                                     xv[:, c, :, m * XG:m * XG + 128],
                                     xv[:, c, :, m * XG:(m + 1) * XG],
                                     start=(c == 0), stop=(c == C - 1),
                                     perf_mode=DR)
                nc.vector.tensor_tensor_reduce(
                    out=gsc_t[:, m * 128:(m + 1) * 128], in0=g_ps[:, 0:128],
                    in1=idb_t[:], scale=1.0, scalar=0.0,
                    op0=Alu.mult, op1=Alu.add, accum_out=ssT_t[:, m:m + 1])
                nc.scalar.activation(graw_t[:, m * G:(m + 1) * G],
                                     g_ps[:, 128:128 + G], Act.Copy)
            lnss_t = work.tile([128, MT], f32, name="lnss")
            nc.scalar.activation(lnss_t[:], ssT_t[:], Act.Ln)
            # inv_m = 1/(||x_m|| * 1024)  (x scaled by 8, W by 128)
            nc.scalar.activation(inv_t[:], lnss_t[:], Act.Exp,
                                 scale=-0.5, bias=cd_t[:, 264:265])
            nc.vector.tensor_scalar_mul(ninv_t[:], inv_t[:], -1.0)
            # gates: sg = 1/(1+exp(-(raw*inv + bg)))
            for m in range(MT):
                nc.vector.scalar_tensor_tensor(
                    out=gt_t[:, m * G:(m + 1) * G],
                    in0=graw_t[:, m * G:(m + 1) * G],
                    scalar=ninv_t[:, m:m + 1], in1=cd_t[:, 256:264],
                    op0=Alu.mult, op1=Alu.add)
            ge_t = work.tile([128, MT * G], f32, name="ge")
            nc.scalar.activation(ge_t[:], gt_t[:], Act.Exp)
            nc.vector.tensor_scalar_add(ge_t[:], ge_t[:], 1.0)
            nc.vector.reciprocal(sg_t[:], ge_t[:])
            # per-drain scales sg*inv, and the sg cols of yT
            for m in range(MT):
                nc.vector.tensor_scalar_mul(
                    sgi_t[:, m * G:(m + 1) * G], sg_t[:, m * G:(m + 1) * G],
                    inv_t[:, m:m + 1])
                nc.vector.tensor_copy(
                    yT3[m][:, :, D + 1:D + 2],
                    sg_t[:, m * G:(m + 1) * G].rearrange("p (g c) -> p g c", c=1))

            # ---------------- fused gk + y chains ----------------
            def chain(j, m, drain):
                ps = psm.tile([128, 512], f32, name="mm_ps", tag="mm", bufs=4)
                for c in range(C):
                    nc.tensor.matmul(ps[:],
                                     xv[:, c, :, m * XG:m * XG + 128],
                                     wv[j][:, c, :, :],
                                     start=(c == 0), stop=(c == C - 1),
                                     perf_mode=DR)
                drain(ps)

            # gk slices (w cols 2048..3071): exp drains on scalar engine
            for j in (4, 5):
                for m in range(MT):
                    h = j - 4
                    def gk_drain(ps, m=m, h=h):
                        nc.scalar.activation(
                            ex_t[m][:, h * 512:(h + 1) * 512], ps[:], Act.Exp,
                            scale=inv_t[:, m:m + 1])
                    chain(j, m, gk_drain)

            # y slices: yT g-block = ps * (sg_g*inv)  (bf16, bias-free)
            def y_drain(ps, j, m):
                for half in range(2):
                    g = 2 * j + half
                    src = ps[:, half * D:(half + 1) * D]
                    dst = yT3[m][:, g, 0:D]
                    sc = sgi_t[:, m * G + g:m * G + g + 1]
                    if m >= 2:
                        nc.scalar.activation(dst, src, Act.Copy, scale=sc)
                    else:
                        nc.vector.tensor_scalar_mul(dst, src, sc)

            for j in (0, 1, 2, 3):
                for m in range(MT):
                    chain(j, m, lambda ps, j=j, m=m: y_drain(ps, j, m))

            # ---------------- VLAD einsum (per-group accumulators) ----------
            # vd_g = [sum_m ex*sg*yhat | se | Sg];  V += (1/se_g) * vd_g
            V_t = fin.tile([128, BW], f32, name="V")
            for g in range(G):
                vd = psm.tile([128, BW], f32, name=f"vd{g}", tag="vd", bufs=2)
                for m in range(MT):
                    nc.tensor.matmul(vd[:],
                                     ex_t[m][:, g * KC:(g + 1) * KC],
                                     yT3[m][:, g, :],
                                     start=(m == 0), stop=(m == MT - 1))
                nc.vector.reciprocal(iseT_t[:, g:g + 1], vd[:, D:D + 1])
                if g == 0:
                    nc.vector.tensor_scalar_mul(V_t[:], vd[:], iseT_t[:, 0:1])
                else:
                    nc.vector.scalar_tensor_tensor(
                        out=V_t[:], in0=vd[:], scalar=iseT_t[:, g:g + 1],
                        in1=V_t[:], op0=Alu.mult, op1=Alu.add)
                # isg_g = Sg / se_g  (for the b_inp correction)
                nc.vector.tensor_mul(isg_t[:, g:g + 1], vd[:, D + 1:D + 2],
                                     iseT_t[:, g:g + 1])

            # bias correction via PE (full fp32): Vc[k,d] = sum_g isg[k,g]*binp[g,d]
            isgT_ps = psm.tile([G, 128], f32, name="isgT", tag="tp", bufs=2)
            nc.tensor.transpose(isgT_ps[:], isg_t[:], idf_t[:])
            isgT_sb = fin.tile([G, 128], f32, name="isgTs")
            nc.scalar.activation(isgT_sb[:], isgT_ps[:], Act.Copy)
            Vc_ps = psm.tile([128, D], f32, name="Vc_ps", tag="tp", bufs=2)
            nc.tensor.matmul(Vc_ps[:], isgT_sb[:], binpm_t[:],
                             start=True, stop=True)

            # vlad = V[:, :D] + Vc + T*(-cent)   (T = V col D+1)
            vlad_t = fin.tile([128, D], f32, name="vlad")
            nc.vector.scalar_tensor_tensor(
                out=vlad_t[:], in0=cd_t[:, 0:D], scalar=V_t[:, D + 1:D + 2],
                in1=V_t[:, 0:D], op0=Alu.mult, op1=Alu.add)
            nc.vector.tensor_add(vlad_t[:], vlad_t[:], Vc_ps[:])
            sq_t = fin.tile([128, D], f32, name="sq")
            ss2_t = fin.tile([128, 1], f32, name="ss2")
            nc.vector.tensor_tensor_reduce(
                out=sq_t[:], in0=vlad_t[:], in1=vlad_t[:], scale=1.0,
                scalar=0.0, op0=Alu.mult, op1=Alu.add, accum_out=ss2_t[:])
            lnv_t = fin.tile([128, 1], f32, name="lnv")
            nc.scalar.activation(lnv_t[:], ss2_t[:], Act.Ln)
            rn_t = fin.tile([128, 1], f32, name="rn")
            nc.scalar.activation(rn_t[:], lnv_t[:], Act.Exp,
                                 scale=-0.5, bias=cd_t[:, 265:266])
            out_t = fin.tile([128, D], f32, name="out")
            nc.vector.tensor_scalar_mul(out_t[:], vlad_t[:], rn_t[:, 0:1])
            nc.sync.dma_start(out=out_d[:], in_=out_t[:])

    nc.compile()
    return nc


def _get_nc():
    if "nc" not in _cache:
        _cache["nc"] = _build_nc()
    return _cache["nc"]


def _pack_rows(a):
    """[1024, w] row-major -> [128, (c s w)] DoubleRow chunk-pair layout."""
    w = a.shape[1]
    return np.ascontiguousarray(
        a.reshape(C, 2, 128, w).transpose(2, 0, 1, 3).reshape(128, C * 2 * w))


def kernel(x, W_inp, b_inp, W_g, b_g, W_gk, b_gk, centroids):
    from concourse.bass_utils import run_bass_kernel_spmd
    import ml_dtypes

    nc = _get_nc()

    x = np.asarray(x, dtype=np.float32)
    X = x.reshape(8, 8, N, 64).transpose(0, 2, 1, 3).reshape(8, N, M)

    Wi = np.asarray(W_inp, np.float32)                     # [2048, 1024]
    Wc = np.asarray(W_gk, np.float32) @ Wi                 # [1024, 1024]
    Wg = np.asarray(W_g, np.float32) @ Wi                  # [8, 1024]
    W_all = np.zeros((N, 3072), np.float32)
    W_all[:, 0:EN] = Wi.T
    W_all[:, EN:EN + G * KC] = Wc.T
    W8 = (W_all * SW).astype(ml_dtypes.float8_e4m3)
    wd = np.concatenate(
        [_pack_rows(W8[:, j * 512:(j + 1) * 512]) for j in range(6)], axis=1)

    wg8 = np.zeros((N, 64), np.float32)
    wg8[:, 0:G] = Wg.T
    wg8 = (wg8 * SW).astype(ml_dtypes.float8_e4m3)

    bg = (np.asarray(b_g, np.float64)
          + np.asarray(W_g, np.float64) @ np.asarray(b_inp, np.float64)
          ).astype(np.float32)
    binpm = np.ascontiguousarray(np.asarray(b_inp, np.float32).reshape(G, D))
    cd = np.zeros((128, 272), np.float32)
    cd[:, 0:D] = -np.asarray(centroids, np.float32)
    cd[:, 256:264] = -bg[None, :]
    cd[:, 264] = -LN128
    cd[:, 265] = -0.5 * LN128
    idf = np.eye(128, dtype=np.float32)
    idb = np.eye(128, dtype=ml_dtypes.bfloat16)

    in_maps = []
    for b in range(8):
        X8 = (X[b] * SX).astype(ml_dtypes.float8_e4m3)
        # augmented x: per (c,s) row-chunk, 4 groups of [128 x-cols | 64 wg]
        xa = np.zeros((C, 2, 128, MT * XG), dtype=ml_dtypes.float8_e4m3)
        Xr = X8.reshape(C, 2, 128, M)
        Wr = wg8.reshape(C, 2, 128, 64)
        for m in range(MT):
            xa[:, :, :, m * XG:m * XG + 128] = Xr[:, :, :, m * 128:(m + 1) * 128]
            xa[:, :, :, m * XG + 128:(m + 1) * XG] = Wr
        xd = np.ascontiguousarray(
            xa.transpose(2, 0, 1, 3).reshape(128, C * 2 * MT * XG))
        in_maps.append({
            "x": xd, "w": wd, "binpm": binpm, "cd": cd,
            "idf": idf, "idb": idb,
        })

    trace = os.environ.get("KERNEL_TRACE") == "1"
    r = run_bass_kernel_spmd(nc, in_maps, core_ids=list(range(8)), trace=trace)
    _cache["last_results"] = r
    return np.stack([r.results[b]["out"].reshape(KC * D)
                     for b in range(8)]).astype(np.float32)
